# revision 10
# baseline (speedup 1.0000x reference)
"""Bass/Trainium2 kernel for nn_FC_Classifier (box-pooled FC classifier), v6.

Math: pred[n,k] = (1/area_n) * sum_{(h,w) in box_n} (fc_w @ feature_map)[k,h,w] + fc_b[k]

Strategy (8 cores, one chip) -- "no collectives":
  * Image rows sharded across cores (24 rows/core).  Phase 1 projects
    channels 2048->150 with matmuls; per row pair the 384 w-positions are
    packed into 3 full 128-wide stationaries (A=r0 w0:128, B=r1 w0:128,
    C=[r0 w128: | r1 w128:]).  W-cumsum via triangular matmuls (incl. a
    base-partition-64 variant for the C chunk); H-cumsum as a DVE running
    add chain -> unprefixed local integral block L[x, hh, k] in SBUF.
  * As each row's column of L finalizes it is split into bf16 hi/lo
    (mixed-dtype DVE sub), and +-1 one-hot corner-pair matmul tiles for
    that row run immediately, interleaved with phase 1 on TensorE.  Tiles
    whose anchors provably don't need the lo term (area-based bound) run
    hi-only; unsafe anchors are sorted into the tail tiles of each group.
  * No collectives, no indirect DMA.  Each core returns pair values + its
    block column totals T; the host combines pair1/pair0 with the
    exclusive block-prefix of T, scales by 1/area, and adds the bias.
  * DMA split over two HWDGE queues (sync: fm/fcw/tri, scalar: one-hots,
    outputs); fm first-chunk sliced for startup latency.
"""

import numpy as np

DS = 8.0
NCORES = 8
C, H, W, K, N_ANCH = 2048, 192, 192, 150, 16384
HSH = H // NCORES              # 24 image rows per core
XP = 200                       # x range of cumsum output (0..192 used)
XA = W + 1                     # 193 x values of the integral image
CCH = C // 128                 # 16 channel chunks
HQ = 12                        # row pairs per core

LAST_RESULTS = None  # BassKernelResults of the most recent run (for test.py)

_NC_CACHE = {}


def _chunks(total, size):
    return [(o, min(size, total - o)) for o in range(0, total, size)]


def _box_indices_np(anchors, scale, h, w):
    # exact replica of reference._box_indices in numpy f32
    a = anchors.astype(np.float32) / np.float32(DS)
    x0 = (a[:, 0] * scale[1]).astype(np.int32)
    x1 = (a[:, 1] * scale[1]).astype(np.int32)
    y0 = (a[:, 2] * scale[0]).astype(np.int32)
    y1 = (a[:, 3] * scale[0]).astype(np.int32)
    eqy = y0 == y1
    y0, y1 = (
        np.where(eqy & (y0 != 0), y0 - 1, y0),
        np.where(eqy & (y0 == 0), y1 + 1, y1),
    )
    eqx = x0 == x1
    x0, x1 = (
        np.where(eqx & (x0 != 0), x0 - 1, x0),
        np.where(eqx & (x0 == 0), x1 + 1, x1),
    )
    y0, y1 = np.clip(y0, 0, h), np.clip(y1, 0, h)
    x0, x1 = np.clip(x0, 0, w), np.clip(x1, 0, w)
    return x0, x1, y0, y1


def _build_nc(TL):
    """Build the SPMD Bass program (identical on all 8 cores).

    TL: list of hh (local image row) per one-hot tile, sorted ascending.
    """
    from concourse import bacc, mybir, tile

    f32 = mybir.dt.float32
    bf16 = mybir.dt.bfloat16

    xch = _chunks(XP, 128)         # [(0,128),(128,72)]   x partition chunks
    NTT = len(TL)

    nc = bacc.Bacc("TRN2", target_bir_lowering=False, debug=False,
                   num_devices=NCORES)
    # host-swizzled fm: [p, hq, cc, w'] with w' = [A(128) B(128) C(128)]
    fm = nc.dram_tensor("fm", [128, HQ, CCH, 384], bf16, kind="ExternalInput").ap()
    fcw = nc.dram_tensor("fcw", [128, CCH, K], bf16, kind="ExternalInput").ap()
    # tri rows 0:192 = lower-tri cumsum matrix; rows 192:320 = triE
    # (rows 64:128 of it hold tri rows 128:192, for the base-64 matmul)
    trib = nc.dram_tensor("trib", [320, XP], bf16, kind="ExternalInput").ap()
    oh0 = nc.dram_tensor("oh0", [128, NTT, 128], bf16, kind="ExternalInput").ap()
    oh1 = nc.dram_tensor("oh1", [65, NTT, 128], bf16, kind="ExternalInput").ap()
    pp = nc.dram_tensor("pp", [NTT * 128, K], f32, kind="ExternalOutput").ap()
    tout = nc.dram_tensor("tout", [XA, K], f32, kind="ExternalOutput").ap()

    NF = HSH * K
    GRP = 8
    ppv = pp.rearrange("(t s) k -> s t k", s=128)

    tiles_of = [[] for _ in range(HSH)]
    for t, (hh, nmm) in enumerate(TL):
        tiles_of[hh].append(t)

    with tile.TileContext(nc) as tc:
        with (
            tc.tile_pool(name="constp", bufs=1) as constp,
            tc.tile_pool(name="fmp", bufs=2) as fmp,
            tc.tile_pool(name="gp", bufs=3) as gp,
            tc.tile_pool(name="qp", bufs=1) as qp,
            tc.tile_pool(name="prjp", bufs=4, space="PSUM") as prjp,
            tc.tile_pool(name="trip", bufs=1, space="PSUM") as trip,
            tc.tile_pool(name="pairp", bufs=2, space="PSUM") as pairp,
            tc.tile_pool(name="ohp", bufs=3) as ohp,
            tc.tile_pool(name="obp", bufs=4) as obp,
        ):
            # ---- first fm slices early (startup latency) -------------------
            CW = CCH * 384
            fmv = fm.rearrange("p hq cc w -> p (hq cc w)")
            fm_groups = [(0, 1), (1, 2), (2, 4), (4, 6), (6, 8), (8, 10), (10, 12)]
            fmh0 = fmp.tile([128, CW], bf16, tag="fmh0", name="fmh0")
            nc.sync.dma_start(fmh0[:, 0:4 * 384], fmv[:, 0:4 * 384])

            fcw_sb = constp.tile([128, CCH * K], bf16, tag="fcw", name="fcw_sb")
            fcwv = fcw.rearrange("p cc k -> p (cc k)")
            nc.sync.dma_start(fcw_sb[:, 0:4 * K], fcwv[:, 0:4 * K])
            for q in range(1, 4):
                nc.sync.dma_start(fmh0[:, q * 4 * 384:(q + 1) * 4 * 384],
                                  fmv[:, q * 4 * 384:(q + 1) * 4 * 384])
                nc.sync.dma_start(fcw_sb[:, q * 4 * K:(q + 1) * 4 * K],
                                  fcwv[:, q * 4 * K:(q + 1) * 4 * K])

            tri0 = constp.tile([128, XP], bf16, tag="tri0", name="tri0")
            nc.sync.dma_start(tri0[:], trib[0:128, :])
            tri1 = constp.tile([64, XP], bf16, tag="tri1", name="tri1")
            nc.sync.dma_start(tri1[:], trib[128:192, :])
            triE = constp.tile([128, XP], bf16, tag="triE", name="triE")
            nc.sync.dma_start(triE[:], trib[192:320, :])

            Qc = [qp.tile([sz, NF], f32, tag=f"Qc{j}", name=f"Qc{j}")
                  for j, (off, sz) in enumerate(xch)]
            Shi = [qp.tile([sz, NF], bf16, tag=f"Shi{j}", name=f"Shi{j}")
                   for j, (off, sz) in enumerate(xch)]
            Slo = [qp.tile([sz, NF], bf16, tag=f"Slo{j}", name=f"Slo{j}")
                   for j, (off, sz) in enumerate(xch)]

            OGRP = 16
            ohsb = [None] * ((NTT + OGRP - 1) // OGRP)

            def prefetch_oh(gi):
                g = min(OGRP, NTT - gi * OGRP)
                t0 = ohp.tile([128, g * 128], bf16, tag="ohsb0", name=f"oh0_{gi}")
                nc.scalar.dma_start(
                    t0[:].rearrange("p (t s) -> p t s", t=g),
                    oh0[:, gi * OGRP:gi * OGRP + g])
                t1 = ohp.tile([65, g * 128], bf16, tag="ohsb1", name=f"oh1_{gi}")
                nc.scalar.dma_start(
                    t1[:].rearrange("p (t s) -> p t s", t=g),
                    oh1[:, gi * OGRP:gi * OGRP + g])
                ohsb[gi] = (t0, t1)

            prefetch_oh(0)
            if len(ohsb) > 1:
                prefetch_oh(1)

            # global 8-tile output staging: flush one DMA per 8 tiles
            ob_state = [None, 0]       # [tile, count]; tiles are contiguous in t

            def flush_ob(t_end):
                ob, cnt = ob_state
                if ob is not None and cnt > 0:
                    nc.scalar.dma_start(
                        ppv[:, t_end - cnt:t_end],
                        ob[:, 0:cnt * K].rearrange("s (t k) -> s t k", t=cnt))
                ob_state[0], ob_state[1] = None, 0

            def pair_tile(t):
                hh, nmm = TL[t]
                gi, ti = t // OGRP, t % OGRP
                if ti == 0 and gi + 2 < len(ohsb) and ohsb[gi + 2] is None:
                    prefetch_oh(gi + 2)
                ps = pairp.tile([128, K], f32, tag="pair", name=f"pair{t}")
                Ss = (Shi, Slo) if nmm == 4 else (Shi,)
                first = True
                for xj, (xoff, xsz) in enumerate(xch):
                    ssz = min(xsz, XA - xoff)
                    lhs = ohsb[gi][xj][0:ssz, ti * 128:(ti + 1) * 128]
                    for S in Ss:
                        nc.tensor.matmul(
                            ps[:], lhsT=lhs,
                            rhs=S[xj][0:ssz, hh * K:(hh + 1) * K],
                            start=first,
                            stop=(xj == len(xch) - 1 and S is Ss[-1]),
                        )
                        first = False
                if ob_state[0] is None:
                    ob_state[0] = obp.tile([128, 8 * K], f32, tag="ob",
                                           name=f"ob{t}")
                nc.scalar.copy(ob_state[0][:, ob_state[1] * K:(ob_state[1] + 1) * K],
                               ps[:])
                ob_state[1] += 1
                if ob_state[1] == 8:
                    flush_ob(t + 1)

            # ---- phase 1 + interleaved split/pair tiles --------------------
            for gi, (ga, gb) in enumerate(fm_groups):
              if gi == 0:
                  fmt = fmh0
              else:
                  fmt = fmp.tile([128, (gb - ga) * CW], bf16, tag="fmh",
                                 name=f"fmh{gi}")
                  nc.sync.dma_start(fmt[:], fmv[:, ga * CW:gb * CW])
              for hq in range(ga, gb):
                hoff = (hq - ga) * CW
                # projection: 3 stationaries [A|B|C], one PSUM bank each;
                # cc outer so the stream follows DMA arrival order
                gt = gp.tile([128, 3 * K], bf16, tag="gt", name="gt")
                pss = [prjp.tile([128, K], f32, tag="prj", name=f"prj{j}")
                       for j in range(3)]
                for cc in range(CCH):
                    for j in range(3):
                        o = hoff + cc * 384 + j * 128
                        nc.tensor.matmul(
                            pss[j][:],
                            lhsT=fmt[:, o: o + 128],
                            rhs=fcw_sb[:, cc * K:(cc + 1) * K],
                            start=(cc == 0), stop=(cc == CCH - 1),
                        )
                for j in range(3):
                    nc.scalar.copy(gt[:, j * K:(j + 1) * K], pss[j][:])
                for hr in range(2):
                    h = hq * 2 + hr
                    for xj, (xoff, xsz) in enumerate(xch):
                        qs = trip.tile([xsz, K], f32, tag=f"tq{xj}", name="qs")
                        nc.tensor.matmul(
                            qs[:],
                            lhsT=tri0[:, xoff:xoff + xsz],
                            rhs=gt[:, hr * K:(hr + 1) * K],
                            start=True, stop=False,
                        )
                        if hr == 0:
                            nc.tensor.matmul(
                                qs[:],
                                lhsT=tri1[:, xoff:xoff + xsz],
                                rhs=gt[0:64, 2 * K:3 * K],
                                start=False, stop=True,
                            )
                        else:
                            nc.tensor.matmul(
                                qs[:],
                                lhsT=triE[64:128, xoff:xoff + xsz],
                                rhs=gt[64:128, 2 * K:3 * K],
                                start=False, stop=True,
                            )
                        if h == 0:
                            nc.vector.tensor_copy(Qc[xj][:, 0:K], qs[:])
                        else:
                            nc.vector.tensor_add(
                                Qc[xj][:, h * K:(h + 1) * K], qs[:],
                                Qc[xj][:, (h - 1) * K:h * K])
                    # split row h into bf16 hi/lo as soon as it is final
                    for xj in range(len(xch)):
                        cs = slice(h * K, (h + 1) * K)
                        nc.vector.tensor_copy(Shi[xj][:, cs], Qc[xj][:, cs])
                        nc.vector.tensor_sub(Slo[xj][:, cs], Qc[xj][:, cs],
                                             Shi[xj][:, cs])
                    # one-hot pair tiles for this row
                    for t in tiles_of[h]:
                        pair_tile(t)

            flush_ob(NTT)

            # ---- block totals out ------------------------------------------
            for xj, (xoff, xsz) in enumerate(xch):
                ssz = min(xsz, XA - xoff)
                nc.scalar.dma_start(tout[xoff:xoff + ssz, :],
                                    Qc[xj][0:ssz, (HSH - 1) * K:HSH * K])

    nc.compile()
    return nc


def _get_nc(TL):
    key = tuple(TL)
    if key not in _NC_CACHE:
        _NC_CACHE[key] = _build_nc(list(TL))
    return _NC_CACHE[key]


def _prepare(feature_map, scale, anchors, fc_w, anchor_num):
    """Host-side prep: swizzle fm, tri, per-core one-hot tiles + slot maps."""
    import ml_dtypes
    bf = ml_dtypes.bfloat16

    N = int(anchor_num)
    assert N == N_ANCH, N
    anchors = np.asarray(anchors, dtype=np.float32)[:N]
    x0, x1, y0, y1 = _box_indices_np(anchors, np.asarray(scale, np.float32), H, W)
    area = np.maximum((y1 - y0) * (x1 - x0), 1).astype(np.float32)

    b1, hh1 = (y1 - 1) // HSH, (y1 - 1) % HSH
    zero0 = y0 == 0
    y0c = np.maximum(y0, 1)
    b0, hh0 = (y0c - 1) // HSH, (y0c - 1) % HSH

    # suffix order: rows processed 23..0; device column c = 22 - hh holds
    # the suffix sum over original rows >= hh+1.  hh==23 needs no tile.
    def tiles_needed(bb, hh, mask):
        cnt = np.zeros((NCORES, HSH), dtype=np.int64)
        np.add.at(cnt, (bb[mask], hh[mask]), 1)
        return np.ceil(cnt.max(axis=0) / 128).astype(int)

    nt1 = tiles_needed(b1, hh1, hh1 <= 22)
    nt0 = tiles_needed(b0, hh0, ~zero0 & (hh0 <= 22))
    TLh = []
    tiles_for = {}
    for c in range(23):
        h = 22 - c
        tiles_for[(1, h)] = list(range(len(TLh), len(TLh) + nt1[h]))
        TLh.extend([c] * nt1[h])
        tiles_for[(0, h)] = list(range(len(TLh), len(TLh) + nt0[h]))
        TLh.extend([c] * nt0[h])
    NTT = len(TLh)

    # bf16-lo need: skip the lo matmuls when the dropped term is provably
    # small; unsafe anchors are sorted to the tail tiles of each group
    def lo_needed(hh, xa, xb):
        hs = (23 - hh).astype(np.float64)          # suffix length
        bound = (2.0 ** -9) * 4.0 * (np.sqrt(hs * xa) + np.sqrt(hs * xb))
        return bound > 0.012 * area

    lo1 = lo_needed(hh1, x1, x0)
    lo0 = lo_needed(hh0, x1, x0)

    oh = np.zeros((NCORES, NTT, 2, 128, 128), dtype=np.int8)
    slot1 = np.full(N, -1, dtype=np.int64)
    slot0 = np.full(N, -1, dtype=np.int64)
    tile_exact = np.zeros(NTT, dtype=bool)

    def assign(pid, bb, hh, mask, lo, slot_out):
        for core in range(NCORES):
            sel = mask & (bb == core)
            for h in range(23):
                safe = np.nonzero(sel & (hh == h) & ~lo)[0]
                unsafe = np.nonzero(sel & (hh == h) & lo)[0]
                idxs = np.concatenate([safe, unsafe])
                ts = tiles_for[(pid, h)]
                assert len(idxs) <= len(ts) * 128, (pid, core, h, len(idxs))
                for j, n in enumerate(idxs):
                    t = ts[j // 128]
                    s = j % 128
                    slot_out[n] = t * 128 + s
                    if lo[n]:
                        tile_exact[t] = True
                    xa, xb = x1[n], x0[n]
                    oh[core, t, xa // 128, xa % 128, s] += 1
                    oh[core, t, xb // 128, xb % 128, s] -= 1

    assign(1, b1, hh1, hh1 <= 22, lo1, slot1)
    assign(0, b0, hh0, ~zero0 & (hh0 <= 22), lo0, slot0)
    TL = [(h, 4 if tile_exact[t] else 2) for t, h in enumerate(TLh)]

    fcwT = np.ascontiguousarray(fc_w.T.astype(bf))                 # [C, K]
    fcw_in = np.ascontiguousarray(
        fcwT.reshape(CCH, 128, K).transpose(1, 0, 2))
    tri = np.zeros((W, XP), dtype=np.float32)
    for x in range(1, W + 1):
        tri[0:x, x] = 1.0
    trib = np.zeros((320, XP), dtype=np.float32)
    trib[0:192] = tri
    trib[256:320] = tri[128:192]          # triE rows 64:128
    trib = trib.astype(bf)

    in_maps = []
    for i in range(NCORES):
        fm_i = feature_map[:, i * HSH:(i + 1) * HSH, :][:, ::-1, :].astype(bf)
        fm_i = fm_i.reshape(CCH, 128, HQ, 2, W)
        fmA = fm_i[:, :, :, 0, 0:128]
        fmB = fm_i[:, :, :, 1, 0:128]
        fmC = np.concatenate([fm_i[:, :, :, 0, 128:192],
                              fm_i[:, :, :, 1, 128:192]], axis=3)
        fm_p = np.concatenate([fmA, fmB, fmC], axis=3)   # [CCH,128,HQ,384]
        fm_p = fm_p.transpose(1, 2, 0, 3)                # [128,HQ,CCH,384]
        oh_i = oh[i].transpose(2, 0, 1, 3).astype(bf)    # [xr, t, c, s]
        in_maps.append({
            "fm": np.ascontiguousarray(fm_p),
            "fcw": fcw_in,
            "trib": trib,
            "oh0": np.ascontiguousarray(oh_i[:, :, 0, :]),
            "oh1": np.ascontiguousarray(oh_i[0:65, :, 1, :]),
        })
    meta = dict(x0=x0, x1=x1, b1=b1, b0=b0, zero0=zero0,
                slot1=slot1, slot0=slot0, area=area)
    return in_maps, meta, TL


def kernel(**inputs):
    global LAST_RESULTS
    feature_map = np.asarray(inputs["feature_map"], dtype=np.float32)
    scale = np.asarray(inputs["scale"], dtype=np.float32)
    anchors = np.asarray(inputs["anchors"], dtype=np.float32)
    fc_w = np.asarray(inputs["fc_w"], dtype=np.float32)
    fc_b = np.asarray(inputs["fc_b"], dtype=np.float32)
    anchor_num = int(np.asarray(inputs["anchor_num"]))

    import os
    import time
    t0 = time.time()
    in_maps, meta, TL = _prepare(feature_map, scale, anchors, fc_w, anchor_num)
    print(f"[kernel] host prep {time.time() - t0:.1f}s NTT={len(TL)}", flush=True)
    t0 = time.time()
    nc = _get_nc(TL)
    print(f"[kernel] bass build+schedule {time.time() - t0:.1f}s", flush=True)

    from concourse.bass_utils import run_bass_kernel_spmd
    trace = bool(int(os.environ.get("NMS_TRACE", "0")))
    t0 = time.time()
    res = run_bass_kernel_spmd(nc, in_maps, core_ids=list(range(NCORES)),
                               trace=trace)
    print(f"[kernel] compile+run {time.time() - t0:.1f}s", flush=True)
    LAST_RESULTS = res

    # ---- host assembly ----------------------------------------------------
    PPS = np.stack([res.results[i]["pp"] for i in range(NCORES)])
    TS = np.stack([res.results[i]["tout"] for i in range(NCORES)])
    Ppre = np.zeros((NCORES, XA, K), dtype=np.float32)
    np.cumsum(TS[:-1], axis=0, out=Ppre[1:])

    b1, b0 = meta["b1"], meta["b0"]
    x0i, x1i = meta["x0"], meta["x1"]
    zero0 = meta["zero0"]
    slot1, slot0 = meta["slot1"], meta["slot0"]
    S1 = np.where(slot1[:, None] >= 0, PPS[b1, np.maximum(slot1, 0)], 0.0)
    pair1 = (TS[b1, x1i] - TS[b1, x0i]) - S1 + Ppre[b1, x1i] - Ppre[b1, x0i]
    S0 = np.where(slot0[:, None] >= 0, PPS[b0, np.maximum(slot0, 0)], 0.0)
    pair0 = (TS[b0, x1i] - TS[b0, x0i]) - S0 + Ppre[b0, x1i] - Ppre[b0, x0i]
    pair0[zero0] = 0.0
    pred = (pair1 - pair0) / meta["area"][:, None] + fc_b[None, :]
    return pred.astype(np.float32)
